# revision 1
# baseline (speedup 1.0000x reference)
"""Trainium2 Bass kernel: custom inverse STFT (degenerate per-bin rotation +
Hann window + overlap-add + window correction).

Math (matching the reference):
    F[i,k]  = S_real[i,k]*A[k] + S_imag[i,k]*B[k]
      A[k]  = w[k]*(cos(th)-sin(th))/n,  B[k] = -w[k]*(cos(th)+sin(th))/n
    out[t]  = sum_i F[i, t-256*i] / max(corr[t], 1e-8)

Sharding: 8192 frames -> 8 cores x 1024 frames.  Core m owns output blocks
[1024m, 1024m+1024) of 256 samples; it loads 3 extra "halo" frames on the
left so every owned block has all 4 overlapping contributions.  The global
tail (blocks 8192..8194, 768 samples) is reconstructed host-side from the
last 3 frames.

On-chip layout per core: frames interleaved as f = 8p + e (partition p gets 8
consecutive frames, 32KB contiguous DRAM per partition).  Overlap-add is then
free-dim-shifted adds on the DVE.  The per-partition wraparound (blocks whose
contributing frame lives on partition p+1) is produced by a shift-by-one-
partition matmul on the TensorEngine (lhsT = shifted identity) accumulating
the host-computed tail frames for partition 127 via a one-hot K=1 matmul.

Engine budget per core: DVE does Sr*A and the final add + overlap-add;
GPSIMD does Si*B (runs concurrently - fp32 1x DVE ops use its dedicated SBUF
port pair); ACT does the chunk-3 copies; PE does the halo shift.
"""

import numpy as np

import concourse.bass as bass
import concourse.bacc as bacc
import concourse.mybir as mybir
import concourse.tile as tile
from concourse.bass_utils import run_bass_kernel_spmd

F32 = mybir.dt.float32
ALU = mybir.AluOpType

P = 128            # SBUF partitions
G = 8              # frames per partition
FL = 1024          # frame length (== fft length)
FS = 256           # frame step
NF = 8192          # total frames
NCORES = 8
FPC = NF // NCORES          # frames owned per core
ROWS = FPC + 3              # input rows per core (3 left-halo frames)
OUT_LEN = FS * (NF - 1) + FL


def _window32():
    # bit-matches the reference's f32 window computation (cancellation in
    # 0.5-0.5*cos makes the f32 rounding of cos visible at the edges, and the
    # output divides by the overlap-added window — numerator and denominator
    # must use the SAME w values for the edge samples to come out right)
    k = np.arange(FL, dtype=np.float32)
    th = np.float32(2.0 * np.pi) * k / np.float32(FL)
    return (np.float32(0.5) - np.float32(0.5) * np.cos(th)).astype(np.float32)


def _coeffs():
    k = np.arange(FL, dtype=np.float64)
    th = 2.0 * np.pi * k / FL
    w = _window32().astype(np.float64)
    a = (w * (np.cos(th) - np.sin(th)) / FL).astype(np.float32)
    b = (-w * (np.cos(th) + np.sin(th)) / FL).astype(np.float32)
    return a, b


def _window_correction():
    w = _window32()
    corr = np.zeros(OUT_LEN, dtype=np.float32)
    for j in range(4):
        chunk = w[j * FS:(j + 1) * FS]
        view = corr[j * FS:j * FS + NF * FS].reshape(NF, FS)
        view += chunk[None, :]
    return corr


def _shift_weights():
    # [129, 128]: rows 0..127 = shifted identity (w[p, q] = 1 iff p == q+1),
    # row 128 = one-hot selecting output partition 127 (for the tail K=1
    # accumulation matmul)
    w = np.zeros((P + 1, P), dtype=np.float32)
    w[1:P, np.arange(P - 1)] = 0.0  # placeholder, set below
    for q in range(P - 1):
        w[q + 1, q] = 1.0
    w[P, P - 1] = 1.0
    return w


def build_nc():
    nc = bacc.Bacc(trn_type="TRN2", target_bir_lowering=False, debug=False)
    sr_d = nc.dram_tensor("s_real", [ROWS, FL], F32, kind="ExternalInput").ap()
    si_d = nc.dram_tensor("s_imag", [ROWS, FL], F32, kind="ExternalInput").ap()
    ca_d = nc.dram_tensor("coef_a", [FL], F32, kind="ExternalInput").ap()
    cb_d = nc.dram_tensor("coef_b", [FL], F32, kind="ExternalInput").ap()
    ft_d = nc.dram_tensor("f_tail", [3 * FL], F32, kind="ExternalInput").ap()
    sw_d = nc.dram_tensor("shiftw", [(P + 1) * P], F32, kind="ExternalInput").ap()
    out_d = nc.dram_tensor("out_seg", [FPC * FS], F32, kind="ExternalOutput").ap()

    # [128, 8, 1024] views: partition p holds input rows 8p..8p+7
    sr3 = sr_d[0:P * G, :].rearrange("(p g) k -> p g k", p=P)
    si3 = si_d[0:P * G, :].rearrange("(p g) k -> p g k", p=P)
    sw2 = sw_d.rearrange("(p q) -> p q", p=P + 1)
    out2 = out_d.rearrange("(p x) -> p x", p=P)      # [128, 2048]

    with tile.TileContext(nc) as tc:
        with (
            tc.tile_pool(name="const", bufs=1) as cpool,
            tc.tile_pool(name="main", bufs=1) as mpool,
            tc.tile_pool(name="tmp", bufs=3) as tpool,
            tc.tile_pool(name="psum", bufs=1, space="PSUM") as ppool,
        ):
            At = cpool.tile([P, FL], F32, tag="At")
            Bt = cpool.tile([P, FL], F32, tag="Bt")
            S1 = cpool.tile([P, P], F32, tag="S1")       # shifted identity
            E127 = cpool.tile([1, P], F32, tag="E127")   # one-hot row
            Ttl = cpool.tile([1, 3 * FL], F32, tag="Ttl")
            Srt = mpool.tile([P, G * FL], F32, tag="Sr")
            Sit = mpool.tile([P, G * FL], F32, tag="Si")
            Ft = mpool.tile([P, G * FL], F32, tag="F")
            Ot = mpool.tile([P, G * FS], F32, tag="O")
            Hp = ppool.tile([P, 2048], F32, tag="Hp")    # halo via PE, 4 banks

            # constants: coefficient broadcast (step-0 DMA) on the tensor
            # engine's queue so it doesn't delay the input stream on sync
            nc.scalar.dma_start(out=At[:, :], in_=ca_d[None, :].broadcast_to([P, FL]))
            nc.scalar.dma_start(out=Bt[:, :], in_=cb_d[None, :].broadcast_to([P, FL]))
            nc.scalar.dma_start(out=S1[:, :], in_=sw2[0:P, :])
            nc.scalar.dma_start(out=E127[:, :], in_=sw2[P:P + 1, :])
            nc.scalar.dma_start(out=Ttl[:, :], in_=ft_d[None, :])

            # stream input + elementwise F, one e-slice at a time (the DVE is
            # the serial bottleneck; the DMA stream stays ahead of it)
            for e in range(G):
                sl = slice(e * FL, (e + 1) * FL)
                nc.sync.dma_start(out=Srt[:, sl], in_=sr3[:, e, :])
                nc.sync.dma_start(out=Sit[:, sl], in_=si3[:, e, :])
                t = tpool.tile([P, FL], F32, tag="t")
                nc.vector.tensor_tensor(out=t[:, :], in0=Sit[:, sl], in1=Bt[:, :], op=ALU.mult)
                nc.vector.tensor_tensor(out=Ft[:, sl], in0=Srt[:, sl], in1=At[:, :], op=ALU.mult)
                nc.vector.tensor_tensor(out=Ft[:, sl], in0=Ft[:, sl], in1=t[:, :], op=ALU.add)

            Fv = Ft[:, :].rearrange("p (g k) -> p g k", g=G)
            Tv = Ttl[:, :].rearrange("p (g k) -> p g k", g=3)
            Ov = Ot[:, :].rearrange("p (g r) -> p g r", g=G)

            # halo by PE shift: Hp[q, :] = F[q+1, sel] (+ tail for q=127)
            # layout: [0:768]   = frames 0..2 chunk0   (read by d=3)
            #         [1024:1536] = frames 0..1 chunk1 (read by d=2)
            #         [1536:1792] = frame 0 chunk2     (read by d=1)
            mm = [
                (slice(0, 512),      (slice(0, 2), slice(0, FS))),        # f01 c0
                (slice(512, 768),    (slice(2, 3), slice(0, FS))),        # f2  c0
                (slice(1024, 1536),  (slice(0, 2), slice(FS, 2 * FS))),   # f01 c1
                (slice(1536, 1792),  (slice(0, 1), slice(2 * FS, 3 * FS))),  # f0 c2
            ]
            for osl, (gsl, ksl) in mm:
                nc.tensor.matmul(Hp[:, osl], S1[:, :], Fv[:, gsl, ksl],
                                 start=True, stop=False)
                nc.tensor.matmul(Hp[:, osl], E127[:, :], Tv[:, gsl, ksl],
                                 start=False, stop=True)

            # overlap-add in two halves so half A streams out early.
            # out[p, b_e] = sum_d F[p, b_e+d, chunk(3-d)], wrap terms from Hp
            # half A: b_e 0..3 (needs F e <= 6, no wrap)
            nc.scalar.copy(out=Ov[:, 0:4, :], in_=Fv[:, 0:4, 3 * FS:4 * FS])
            for d in (1, 2, 3):
                c = 3 - d
                csl = slice(c * FS, (c + 1) * FS)
                nc.vector.tensor_tensor(
                    out=Ov[:, 0:4, :], in0=Ov[:, 0:4, :],
                    in1=Fv[:, d:4 + d, csl], op=ALU.add)
            nc.sync.dma_start(out=out2[:, 0:4 * FS], in_=Ot[:, 0:4 * FS])

            # half B: b_e 4..7 (wrap terms read PSUM)
            nc.scalar.copy(out=Ov[:, 4:8, :], in_=Fv[:, 4:8, 3 * FS:4 * FS])
            hp_sl = {1: slice(1536, 1792), 2: slice(1024, 1536), 3: slice(0, 768)}
            for d in (1, 2, 3):
                c = 3 - d
                csl = slice(c * FS, (c + 1) * FS)
                nc.vector.tensor_tensor(
                    out=Ov[:, 4:8 - d, :], in0=Ov[:, 4:8 - d, :],
                    in1=Fv[:, 4 + d:8, csl], op=ALU.add)
                # wrap blocks b_e = 8-d..7 <- Hp (flat slices so shapes match)
                osl = slice((8 - d) * FS, 8 * FS)
                nc.vector.tensor_tensor(
                    out=Ot[:, osl], in0=Ot[:, osl],
                    in1=Hp[:, hp_sl[d]], op=ALU.add)
            nc.scalar.dma_start(out=out2[:, 4 * FS:], in_=Ot[:, 4 * FS:])
    nc.compile()
    return nc


_cache = {}


def _get_nc():
    if "nc" not in _cache:
        _cache["nc"] = build_nc()
    return _cache["nc"]


def make_in_maps(S_real, S_imag):
    a, b = _coeffs()
    pad = np.zeros((3, FL), dtype=np.float32)
    sr_pad = np.concatenate([pad, S_real], axis=0)
    si_pad = np.concatenate([pad, S_imag], axis=0)
    shiftw = _shift_weights().reshape(-1)
    in_maps = []
    for m in range(NCORES):
        r0 = m * FPC
        hi = m * FPC + FPC - 3
        # host-computed F for this core's last 3 own frames (feeds partition
        # 127's halo)
        ftl = (S_real[hi:hi + 3] * a[None, :] + S_imag[hi:hi + 3] * b[None, :])
        in_maps.append({
            "s_real": np.ascontiguousarray(sr_pad[r0:r0 + ROWS]),
            "s_imag": np.ascontiguousarray(si_pad[r0:r0 + ROWS]),
            "coef_a": a,
            "coef_b": b,
            "f_tail": np.ascontiguousarray(ftl.reshape(-1)),
            "shiftw": shiftw,
        })
    return in_maps


def assemble_output(S_real, S_imag, segs):
    a, b = _coeffs()
    out = np.zeros(OUT_LEN, dtype=np.float32)
    for m in range(NCORES):
        out[m * FPC * FS:(m + 1) * FPC * FS] = segs[m]

    # global tail: blocks 8192..8194 from the last 3 frames
    hf = (S_real[NF - 3:] * a[None, :] + S_imag[NF - 3:] * b[None, :])
    for t in range(3):
        i = NF - 3 + t
        for j in range(3 - t, 4):
            blk = i + j
            out[blk * FS:(blk + 1) * FS] += hf[t, j * FS:(j + 1) * FS]

    if "corr" not in _cache:
        _cache["corr"] = _window_correction()
    corr = _cache["corr"]
    return out / np.maximum(corr, np.float32(1e-8))


def kernel(S_real, S_imag):
    S_real = np.asarray(S_real, dtype=np.float32)
    S_imag = np.asarray(S_imag, dtype=np.float32)
    in_maps = make_in_maps(S_real, S_imag)
    nc = _get_nc()
    res = run_bass_kernel_spmd(nc, in_maps, list(range(NCORES)))
    segs = [res.results[m]["out_seg"] for m in range(NCORES)]
    return assemble_output(S_real, S_imag, segs)



# revision 7
# speedup vs baseline: 1.7969x; 1.7969x over previous
"""Trainium2 Bass kernel: custom inverse STFT (per-bin rotation + Hann window
+ overlap-add + window correction), fp16 device path.

Math (matching the reference):
    F[i,k] = S_real[i,k]*A[k] + S_imag[i,k]*B[k]
      A[k] = w[k]*(cos(th)-sin(th))/n,  B[k] = -w[k]*(cos(th)+sin(th))/n
    out[t] = sum_i F[i, t-256*i] / max(corr[t], 1e-8)

Sharding: 8192 frames -> 8 cores x 1024 frames.  Each core computes a padded
per-partition overlap-add segment; ALL overlaps (across partitions and across
cores) are resolved on the host by strided adds, so the device kernel has no
halo exchange at all.

Per-core layout: partition p holds frames 8p..8p+7 (16KB fp16 contiguous in
DRAM per partition).  Device pipeline per core:
  - inputs stream in fp16 quarters (2 frames/partition each) on the sync queue
  - DVE computes products P1 = Sr*A~, P2 = Si*B~ (A~/B~ broadcast along the
    frame axis with a stride-0 AP); coefficients are pre-scaled by 256 on the
    host so fp16 products stay in the normal range
  - PE overlap-adds: for each 512-f32 PSUM window of the padded block axis,
    8 identity-lhsT matmuls (4 chunk shifts x 2 tensors) accumulate shifted
    reads of P1/P2 into PSUM; P1/P2 carry 3 zeroed pad frames on each side so
    every matmul in a window covers the identical region
  - ACT copies each finished PSUM window to SBUF as fp16 and the output
    streams out on the scalar queue
Host: assemble per-core [128, 11, 256] padded segments with strided adds,
divide by the precomputed window correction (and the 256 coefficient scale).
"""

import numpy as np

import concourse.bass as bass
import concourse.bacc as bacc
import concourse.mybir as mybir
import concourse.tile as tile
from concourse.bass_utils import run_bass_kernel_spmd

F16 = mybir.dt.float16
F32 = mybir.dt.float32
ALU = mybir.AluOpType

P = 128            # SBUF partitions
G = 8              # frames per partition
FL = 1024          # frame length (== fft length)
FS = 256           # frame step
NF = 8192          # total frames
NCORES = 8
FPC = NF // NCORES          # frames owned per core (1024)
OUT_LEN = FS * (NF - 1) + FL
PAD = 1                     # zero pad frames on each side of product tiles
                            # (all-pad chunk shifts are skipped, so only one
                            # neighbor frame on each side is ever read)
PFR = G + 2 * PAD           # padded frames per partition (14)
NB = G + 3                  # output blocks per partition (11)
OLEN = NB * FS              # 2816 samples per partition
SCALE = 2.0 ** 15           # host-side coefficient prescale: keeps the tiny
                            # edge coefficients (|A[1]|~9e-9, amplified 1e5x by
                            # the window correction) in fp16 NORMAL range
NQ = 4                      # input quarters (2 frames/partition each)


def _window32():
    # bit-matches the reference's f32 window computation
    k = np.arange(FL, dtype=np.float32)
    th = np.float32(2.0 * np.pi) * k / np.float32(FL)
    return (np.float32(0.5) - np.float32(0.5) * np.cos(th)).astype(np.float32)


def _coeffs16():
    k = np.arange(FL, dtype=np.float64)
    th = 2.0 * np.pi * k / FL
    w = _window32().astype(np.float64)
    a = w * (np.cos(th) - np.sin(th)) / FL * SCALE
    b = -w * (np.cos(th) + np.sin(th)) / FL * SCALE
    return a.astype(np.float16), b.astype(np.float16)


def _window_correction():
    w = _window32()
    corr = np.zeros(OUT_LEN, dtype=np.float32)
    for j in range(4):
        chunk = w[j * FS:(j + 1) * FS]
        view = corr[j * FS:j * FS + NF * FS].reshape(NF, FS)
        view += chunk[None, :]
    return corr


def build_nc():
    nc = bacc.Bacc(trn_type="TRN2", target_bir_lowering=False, debug=False)
    sr_d = nc.dram_tensor("s_real", [FPC, FL], F16, kind="ExternalInput").ap()
    si_d = nc.dram_tensor("s_imag", [FPC, FL], F16, kind="ExternalInput").ap()
    ca_d = nc.dram_tensor("coef_a", [FL], F16, kind="ExternalInput").ap()
    cb_d = nc.dram_tensor("coef_b", [FL], F16, kind="ExternalInput").ap()
    id_d = nc.dram_tensor("ident", [P, P], F16, kind="ExternalInput").ap()
    out_d = nc.dram_tensor("out_seg", [P, OLEN], F16, kind="ExternalOutput").ap()

    sr3 = sr_d.rearrange("(p g) k -> p g k", p=P)
    si3 = si_d.rearrange("(p g) k -> p g k", p=P)

    with tile.TileContext(nc) as tc:
        with (
            tc.tile_pool(name="const", bufs=1) as cpool,
            tc.tile_pool(name="main", bufs=1) as mpool,
            tc.tile_pool(name="psum", bufs=1, space="PSUM") as ppool,
        ):
            At = cpool.tile([P, FL], F16, tag="At")
            Bt = cpool.tile([P, FL], F16, tag="Bt")
            It = cpool.tile([P, P], F16, tag="It")
            Srt = mpool.tile([P, G * FL], F16, tag="Sr")
            Sit = mpool.tile([P, G * FL], F16, tag="Si")
            P1t = mpool.tile([P, PFR * FL], F16, tag="P1")
            P2t = mpool.tile([P, PFR * FL], F16, tag="P2")
            # per-chunk output tiles and per-window PSUM tiles: distinct tags
            # so the tile framework doesn't serialize independent windows on
            # false whole-tile WAR hazards
            windows = [(0, 2), (2, 4), (4, 6), (6, 8), (8, 10), (10, 11)]
            Otc = [mpool.tile([P, n * FS], F16, tag=f"Oc{i}", name=f"Oc{i}")
                   for i, n in enumerate((4, 4, 3))]
            Opw = [ppool.tile([P, (b1 - b0) * FS], F32, tag=f"Ops{w}",
                              name=f"Ops{w}")
                   for w, (b0, b1) in enumerate(windows[:5])]

            # constants on the scalar (ACT) DMA queue
            nc.scalar.dma_start(out=At[:, :], in_=ca_d[None, :].broadcast_to([P, FL]))
            nc.scalar.dma_start(out=Bt[:, :], in_=cb_d[None, :].broadcast_to([P, FL]))
            nc.scalar.dma_start(out=It[:, :], in_=id_d[:, :])

            # zero the pad frames once on DVE (it is idle until the first
            # input quarter lands anyway)
            for T in (P1t, P2t):
                nc.vector.memset(T[:, 0:PAD * FL], 0.0)
                nc.vector.memset(T[:, (PAD + G) * FL:], 0.0)

            # input stream: quarters of 2 frames/partition, sync (SP) queue
            for q in range(NQ):
                sl = slice(2 * q * FL, (2 * q + 2) * FL)
                nc.sync.dma_start(out=Srt[:, sl], in_=sr3[:, 2 * q:2 * q + 2, :])
                nc.sync.dma_start(out=Sit[:, sl], in_=si3[:, 2 * q:2 * q + 2, :])

            P1v = P1t[:, :].rearrange("p (g c j) -> p g c j", g=PFR, c=4)
            P2v = P2t[:, :].rearrange("p (g c j) -> p g c j", g=PFR, c=4)

            def emit_products(q):
                sl = slice(2 * q * FL, (2 * q + 2) * FL)
                psl = slice((PAD + 2 * q) * FL, (PAD + 2 * q + 2) * FL)
                for S, C, T in ((Srt, At, P1t), (Sit, Bt, P2t)):
                    nc.vector.tensor_tensor(
                        out=T[:, psl].rearrange("p (g k) -> p g k", g=2),
                        in0=S[:, sl].rearrange("p (g k) -> p g k", g=2),
                        in1=C[:, None, :].broadcast_to([P, 2, FL]),
                        op=ALU.mult,
                    )

            def emit_window(w):
                # PE identity matmuls accumulate the chunk shifts x 2 tensors
                # into this window's private PSUM bank, then ACT copies to the
                # right output chunk as fp16.  Shifts whose whole frame range
                # falls in the pads are skipped (they would add zeros).
                b0, b1 = windows[w]
                Ow = Opw[w]
                Ov = Ow[:, :].rearrange("p (b j) -> p b j", b=b1 - b0)
                seq = [(t, c) for t in (0, 1) for c in range(4)
                       if (b1 - c > 0) and (b0 - c < G)]
                for i, (t, c) in enumerate(seq):
                    src = P1v if t == 0 else P2v
                    nc.tensor.matmul(
                        Ov[:, :, :], It[:, :],
                        src[:, b0 - c + PAD:b1 - c + PAD, c, :],
                        start=(i == 0), stop=(i == len(seq) - 1),
                    )
                chunk, coff = divmod(b0, 4)
                nc.scalar.copy(out=Otc[chunk][:, coff * FS:(coff + b1 - b0) * FS],
                               in_=Ow[:, :])

            def emit_window5_dve():
                # block 10 has exactly one real contribution pair (frame 7,
                # chunk 3): a single DVE add, running while PE drains windows
                # 3-4 — shortens the post-product tail
                nc.vector.tensor_tensor(
                    out=Otc[2][:, 2 * FS:3 * FS],
                    in0=P1v[:, G - 1 + PAD, 3, :],
                    in1=P2v[:, G - 1 + PAD, 3, :], op=ALU.add)

            # pipeline: products gate windows; emit in readiness order
            emit_products(0)
            emit_products(1)
            emit_window(0)          # frames <= 1   (q0)
            emit_products(2)
            emit_window(1)          # frames <= 3   (q0,q1)
            emit_products(3)
            emit_window(2)          # frames <= 5   (q2)
            nc.scalar.dma_start(out=out_d[:, 0:4 * FS], in_=Otc[0][:, :])
            emit_window(3)          # frames <= 7   (q3)
            emit_window5_dve()
            emit_window(4)
            nc.scalar.dma_start(out=out_d[:, 4 * FS:8 * FS], in_=Otc[1][:, :])
            nc.scalar.dma_start(out=out_d[:, 8 * FS:], in_=Otc[2][:, :])
    nc.compile()
    return nc


_cache = {}


def _get_nc():
    if "nc" not in _cache:
        _cache["nc"] = build_nc()
    return _cache["nc"]


def make_in_maps(S_real, S_imag):
    a16, b16 = _coeffs16()
    sr16 = S_real.astype(np.float16)
    si16 = S_imag.astype(np.float16)
    ident = np.eye(P, dtype=np.float16)
    in_maps = []
    for m in range(NCORES):
        r0 = m * FPC
        in_maps.append({
            "s_real": np.ascontiguousarray(sr16[r0:r0 + FPC]),
            "s_imag": np.ascontiguousarray(si16[r0:r0 + FPC]),
            "coef_a": a16,
            "coef_b": b16,
            "ident": ident,
        })
    return in_maps


def assemble_output(segs):
    # segs: per core [128, 2816] fp16 padded OA partials
    acc = np.zeros((NF + NB - G, FS), dtype=np.float32)   # [8195, 256]
    for m in range(NCORES):
        seg = segs[m].astype(np.float32).reshape(P, NB, FS)
        accm = acc[FPC * m: FPC * m + FPC + NB - G]       # [1027, 256] view
        for b in range(NB):
            accm[b: b + (P - 1) * G + 1: G] += seg[:, b, :]
    if "corr" not in _cache:
        _cache["corr"] = _window_correction()
    corr = _cache["corr"]
    return (acc.reshape(-1) / (np.maximum(corr, np.float32(1e-8))
                               * np.float32(SCALE))).astype(np.float32)


def kernel(S_real, S_imag):
    S_real = np.asarray(S_real, dtype=np.float32)
    S_imag = np.asarray(S_imag, dtype=np.float32)
    in_maps = make_in_maps(S_real, S_imag)
    nc = _get_nc()
    res = run_bass_kernel_spmd(nc, in_maps, list(range(NCORES)))
    segs = [res.results[m]["out_seg"] for m in range(NCORES)]
    return assemble_output(segs)
